# revision 23
# baseline (speedup 1.0000x reference)
"""COO SpMM (gnn message passing aggregator) on 8 trn2 NeuronCores.

out = A @ x where A is sparse COO (rows sorted): out[r] += vals[e] * x[cols[e]].

Strategy (self-contained; hardcoded for x[50000,128], 800000 edges, 8 cores):
- Destination rows sharded across 8 cores: core c owns rows
  [6272c, 6272c+6272) = 49 blocks of 128 rows (rows >= 50000 dead/trimmed).
- dma_gather uses int16 indices (max 32767 < 50000 rows), so each block's
  edges are split by source column: "low" (col < 32768) and "high". Low
  edges are packed into K_CL 128-edge chunks gathered from x[0:32768];
  high edges into K_CH chunks gathered from a x[32768:] base view. All
  stream entries are valid (padding slots gather row 0 of the view, with
  edge value 0), so num_idxs_reg == num_idxs is a compile-time constant.
- Device, per BPG-block region: one dma_gather per pass pulls the region's
  x rows (512B HBM descriptors) into SBUF tiles
  gtL [128 x BPG*K_CL x 128] / gtH [128 x BPG*K_CH x 128]
  (dst[i%128, i//128] = src[idx_i]).
- Per chunk: DVE builds scatter matrix S[p, j] = vals[p] * (j == localrow[p])
  with one tensor_scalar (is_equal, mult) against an iota row; PE accumulates
  S.T @ gathered into a PSUM tile [128 rows x 128 feat] (fp32, start/stop
  over the block's K_CL + K_CH chunks). ACT drains PSUM -> SBUF, HWDGE DMA
  stores each 128-row block.
- Host concatenates per-core outputs and trims to 50000 rows.
"""

import os
import numpy as np
from contextlib import ExitStack

import concourse.bass as bass
import concourse.tile as tile
from concourse import bacc, mybir
from concourse.bass_utils import run_bass_kernel_spmd

N_NODES = 50000
N_EDGES = 800000
D = 128
NCORES = 8
BLK = 128
NBLK = 49                 # blocks per core
RPC = NBLK * BLK          # 6272 rows per core (8*6272 = 50176 >= 50000)
HI = 32768                # int16 index range split
BPG = 2                   # blocks per gather region
GBUFS = 5                 # gather tile pool buffers

last_exec_ns = None       # stashed by kernel() when profiling is enabled


def _shard(rows, cols, vals):
    """Pack edges into per-core low/high chunk streams.

    Returns (idxL, idxH, lr, valsx, K_CL, K_CH) where
      idxL: [NCORES, 128, NBLK*K_CL*128/16] int16 gather stream (cols < HI)
      idxH: [NCORES, 128, NBLK*K_CH*128/16] int16 gather stream (cols - HI)
      lr/valsx: [NCORES, 128, NBLK*(K_CL+K_CH)] float32 per-slot
                localrow / edge value (chunk-major slot layout)
    """
    core = rows // RPC
    local = rows - core * RPC
    blk = local // BLK
    lr_e = (local - blk * BLK).astype(np.float32)
    low = cols < HI

    key = (core * NBLK + blk) * 2 + (~low).astype(np.int64)
    counts = np.bincount(key, minlength=NCORES * NBLK * 2)
    cl = counts[0::2]
    ch = counts[1::2]
    K_CL = max(1, int(np.ceil(cl.max() / 128)))
    K_CH = max(1, int(np.ceil(ch.max() / 128)))
    K_TOT = K_CL + K_CH
    C = NBLK * K_TOT

    # index within (core, block, low/high) group; edges sorted by rows so
    # key groups are contiguous in a stable argsort of key
    order = np.argsort(key, kind="stable")
    starts = np.zeros(NCORES * NBLK * 2, np.int64)
    np.cumsum(counts[:-1], out=starts[1:])
    j = np.empty(len(rows), np.int64)
    j[order] = np.arange(len(rows)) - starts[key[order]]

    # chunk index within the core (chunk-major slot layout)
    sub = np.where(low, j // 128, K_CL + j // 128)
    chunk = blk * K_TOT + sub
    part = j % 128

    lrs = np.zeros((NCORES, 128, C), np.float32)
    valsx = np.zeros((NCORES, 128, C), np.float32)
    lrs[core, part, chunk] = lr_e
    valsx[core, part, chunk] = vals

    # gather streams: position within pass-P stream of a core =
    # (blk*K_CP + j//128)*128 + j%128 ; padding -> index 0
    SL = NBLK * K_CL * 128
    SH = NBLK * K_CH * 128
    sL = np.zeros((NCORES, SL), np.int16)
    sH = np.zeros((NCORES, SH), np.int16)
    posL = (blk[low] * K_CL + j[low] // 128) * 128 + j[low] % 128
    posH = (blk[~low] * K_CH + j[~low] // 128) * 128 + j[~low] % 128
    sL[core[low], posL] = cols[low].astype(np.int16)
    sH[core[~low], posH] = (cols[~low] - HI).astype(np.int16)

    # [16, S/16] wrap (idx i at [i%16, i//16]), replicated to 128 partitions
    # (each of the 8 GPSIMD Q7 cores reads its own 16-partition window)
    idxL = np.tile(sL.reshape(NCORES, SL // 16, 16).transpose(0, 2, 1),
                   (1, 8, 1)).copy()
    idxH = np.tile(sH.reshape(NCORES, SH // 16, 16).transpose(0, 2, 1),
                   (1, 8, 1)).copy()
    return idxL, idxH, lrs, valsx, K_CL, K_CH


def _build(K_CL, K_CH):
    K_TOT = K_CL + K_CH
    C = NBLK * K_TOT
    SL = NBLK * K_CL * 128
    SH = NBLK * K_CH * 128
    nreg = -(-NBLK // BPG)
    nc = bacc.Bacc("TRN2", target_bir_lowering=False, debug=False,
                   num_devices=NCORES, dynamic_dma_scratch_size=65536,
                   num_swdge_queues=4, detect_race_conditions=False)
    f32 = mybir.dt.float32
    x_ap = nc.dram_tensor("x", [N_NODES, D], f32, kind="ExternalInput").ap()
    iL_ap = nc.dram_tensor("idxL", [128, SL // 16], mybir.dt.int16,
                           kind="ExternalInput").ap()
    iH_ap = nc.dram_tensor("idxH", [128, SH // 16], mybir.dt.int16,
                           kind="ExternalInput").ap()
    lr_ap = nc.dram_tensor("lr", [128, C], f32, kind="ExternalInput").ap()
    nlr_ap = nc.dram_tensor("nlr", [128, C], f32, kind="ExternalInput").ap()
    nvals_ap = nc.dram_tensor("nvals", [128, C], f32, kind="ExternalInput").ap()
    vals_ap = nc.dram_tensor("vals", [128, C], f32, kind="ExternalInput").ap()
    iota_ap = nc.dram_tensor("iota", [128, 129], f32, kind="ExternalInput").ap()
    out_ap = nc.dram_tensor("out", [RPC, D], f32, kind="ExternalOutput").ap()
    out_v = out_ap.rearrange("(b p) d -> b p d", p=128)

    with tile.TileContext(nc) as tc:
        with ExitStack() as ctx:
            pp = ctx.enter_context(tc.tile_pool(name="persist", bufs=1))
            gpl = ctx.enter_context(tc.tile_pool(name="gatherL", bufs=GBUFS))
            gph = ctx.enter_context(tc.tile_pool(name="gatherH", bufs=GBUFS))
            sp = ctx.enter_context(tc.tile_pool(name="sbuild", bufs=22))
            ps = ctx.enter_context(tc.tile_pool(name="psum", bufs=8,
                                                space="PSUM"))
            stg = ctx.enter_context(tc.tile_pool(name="stage", bufs=6))
            tp = ctx.enter_context(tc.tile_pool(name="tsq", bufs=10))

            iL_t = pp.tile([128, SL // 16], mybir.dt.int16)
            nc.sync.dma_start(iL_t[:], iL_ap[:])
            iH_t = pp.tile([128, SH // 16], mybir.dt.int16)
            nc.sync.dma_start(iH_t[:], iH_ap[:])
            lr_t = pp.tile([128, C], f32)
            nc.sync.dma_start(lr_t[:], lr_ap[:])
            nlr_t = pp.tile([128, C], f32)
            nc.sync.dma_start(nlr_t[:], nlr_ap[:])
            nvals_t = pp.tile([128, C], f32)
            nc.sync.dma_start(nvals_t[:], nvals_ap[:])
            vals_t = pp.tile([128, C], f32)
            nc.sync.dma_start(vals_t[:], vals_ap[:])
            iota_t = pp.tile([128, 129], f32)
            nc.sync.dma_start(iota_t[:], iota_ap[:])

            gtsL = [None] * nreg
            gtsH = [None] * nreg
            # per-queue DMA-completion sems: completions within one SWDGE
            # queue are in ring order, so per-queue counts are monotone
            qsems = [nc.alloc_semaphore(f"gq{q}") for q in range(4)]
            qcount = [0, 0, 0, 0]
            # (region) -> [(queue, target_count), (queue, target_count)]
            gtarget = [None] * nreg

            def issue_gathers(r):
                # prepare_only + per-queue trigger: Q7 only writes the ring
                # (descriptor gen), the SDMA engines drain 4 queue rings in
                # parallel; data-completion gating is via per-queue sems
                nblk_r = min(BPG, NBLK - r * BPG)
                nL = nblk_r * K_CL * 128
                nH = nblk_r * K_CH * 128
                gtsL[r] = gpl.tile([128, BPG * K_CL, D], f32, name=f"gtL{r}", tag="gtL")
                gtsH[r] = gph.tile([128, BPG * K_CH, D], f32, name=f"gtH{r}", tag="gtH")
                aL = r * BPG * K_CL * 128 // 16
                aH = r * BPG * K_CH * 128 // 16
                q = r % 4
                nc.gpsimd.dma_gather(
                    out_ap=gtsL[r][:, :nblk_r * K_CL, :],
                    in_ap=x_ap[:],
                    idxs_ap=iL_t[:, aL:aL + nL // 16],
                    num_idxs=nL,
                    num_idxs_reg=nL,
                    elem_size=D,
                    single_packet=False,
                    queue_num=q,
                )
                nc.gpsimd.dma_gather(
                    out_ap=gtsH[r][:, :nblk_r * K_CH, :],
                    in_ap=x_ap[HI:, :],
                    idxs_ap=iH_t[:, aH:aH + nH // 16],
                    num_idxs=nH,
                    num_idxs_reg=nH,
                    elem_size=D,
                    single_packet=False,
                    queue_num=q,
                )
                qcount[q] += 1
                gtarget[r] = (q, 32 * qcount[q])

            def do_block(b):
                r, brel = divmod(b, BPG)
                pt = ps.tile([128, 128], f32)
                for k in range(K_TOT):
                    c = b * K_TOT + k
                    if k < K_CL:
                        rhs = gtsL[r][:, brel * K_CL + k, :]
                    else:
                        rhs = gtsH[r][:, brel * K_CH + (k - K_CL), :]
                    s_t = sp.tile([128, 129], f32)
                    if k % 2 == 0:
                        nc.vector.tensor_scalar(
                            s_t[:], iota_t[:], lr_t[:, c:c + 1],
                            vals_t[:, c:c + 1],
                            mybir.AluOpType.is_equal, mybir.AluOpType.mult)
                    else:
                        # ACT: S = Relu(vals*(1 - (iota - lr)^2)), exact for
                        # integer iota-lr and vals >= 0
                        t_t = tp.tile([128, 129], f32, name=f"t{c}", tag="tsq")
                        nc.scalar.activation(
                            t_t[:], iota_t[:],
                            mybir.ActivationFunctionType.Square,
                            bias=nlr_t[:, c:c + 1])
                        nc.scalar.activation(
                            s_t[:], t_t[:],
                            mybir.ActivationFunctionType.Relu,
                            bias=vals_t[:, c:c + 1],
                            scale=nvals_t[:, c:c + 1])
                    mm = nc.tensor.matmul(pt[:], lhsT=s_t[:, :128], rhs=rhs,
                                          start=(k == 0),
                                          stop=(k == K_TOT - 1))

                ot = stg.tile([128, 128], f32)
                nc.scalar.copy(ot[:], pt[:])
                nc.sync.dma_start(out_v[b], ot[:])

            issue_gathers(0)
            issue_gathers(1)
            issue_gathers(2)
            for r in range(nreg):
                if r + 3 < nreg:
                    issue_gathers(r + 3)
                for brel in range(BPG):
                    b = r * BPG + brel
                    if b < NBLK:
                        do_block(b)

    nc.compile()
    return nc


_CACHE = {}


def kernel(x, vals, rows, cols):
    global last_exec_ns
    x = np.ascontiguousarray(np.asarray(x, dtype=np.float32))
    vals = np.asarray(vals, dtype=np.float32)
    rows = np.asarray(rows, dtype=np.int64)
    cols = np.asarray(cols, dtype=np.int64)
    assert x.shape == (N_NODES, D) and vals.shape == rows.shape == cols.shape \
        == (N_EDGES,)

    idxL, idxH, lrs, valsx, K_CL, K_CH = _shard(rows, cols, vals)

    key = (K_CL, K_CH)
    if key not in _CACHE:
        _CACHE[key] = _build(K_CL, K_CH)
    nc = _CACHE[key]

    iota = np.broadcast_to(np.arange(129, dtype=np.float32), (128, 129)).copy()
    in_maps = [
        {"x": x, "idxL": idxL[c], "idxH": idxH[c],
         "lr": lrs[c], "vals": valsx[c], "nlr": -lrs[c], "nvals": -valsx[c],
         "iota": iota}
        for c in range(NCORES)
    ]

    trace = os.environ.get("KERNEL_PROFILE", "0") == "1"
    res = run_bass_kernel_spmd(nc, in_maps, core_ids=list(range(NCORES)),
                               trace=trace)
    last_exec_ns = res.exec_time_ns

    out = np.concatenate([res.results[c]["out"] for c in range(NCORES)],
                         axis=0)
    return out[:N_NODES]


# revision 24
# speedup vs baseline: 1.3542x; 1.3542x over previous
"""COO SpMM (gnn message passing aggregator) on 8 trn2 NeuronCores.

out = A @ x where A is sparse COO (rows sorted): out[r] += vals[e] * x[cols[e]].

Strategy (self-contained; hardcoded for x[50000,128], 800000 edges, 8 cores):
- Destination rows sharded across 8 cores: core c owns rows
  [6272c, 6272c+6272) = 49 blocks of 128 rows (rows >= 50000 dead/trimmed).
- dma_gather uses int16 indices (max 32767 < 50000 rows), so each block's
  edges are split by source column: "low" (col < 32768) and "high". Low
  edges are packed into K_CL 128-edge chunks gathered from x[0:32768];
  high edges into K_CH chunks gathered from a x[32768:] base view. All
  stream entries are valid (padding slots gather row 0 of the view, with
  edge value 0), so num_idxs_reg == num_idxs is a compile-time constant.
- Device, per BPG-block region: one dma_gather per pass pulls the region's
  x rows (512B HBM descriptors) into SBUF tiles
  gtL [128 x BPG*K_CL x 128] / gtH [128 x BPG*K_CH x 128]
  (dst[i%128, i//128] = src[idx_i]).
- Per chunk: DVE builds scatter matrix S[p, j] = vals[p] * (j == localrow[p])
  with one tensor_scalar (is_equal, mult) against an iota row; PE accumulates
  S.T @ gathered into a PSUM tile [128 rows x 128 feat] (fp32, start/stop
  over the block's K_CL + K_CH chunks). ACT drains PSUM -> SBUF, HWDGE DMA
  stores each 128-row block.
- Host concatenates per-core outputs and trims to 50000 rows.
"""

import os
import numpy as np
from contextlib import ExitStack

import concourse.bass as bass
import concourse.tile as tile
from concourse import bacc, mybir
from concourse.bass_utils import run_bass_kernel_spmd

N_NODES = 50000
N_EDGES = 800000
D = 128
NCORES = 8
BLK = 128
NBLK = 49                 # blocks per core
RPC = NBLK * BLK          # 6272 rows per core (8*6272 = 50176 >= 50000)
HI = 32768                # int16 index range split
BPG = 1                   # blocks per gather region
GBUFS = 10                # gather tile pool buffers

last_exec_ns = None       # stashed by kernel() when profiling is enabled


def _shard(rows, cols, vals):
    """Pack edges into per-core low/high chunk streams.

    Returns (idxL, idxH, lr, valsx, K_CL, K_CH) where
      idxL: [NCORES, 128, NBLK*K_CL*128/16] int16 gather stream (cols < HI)
      idxH: [NCORES, 128, NBLK*K_CH*128/16] int16 gather stream (cols - HI)
      lr/valsx: [NCORES, 128, NBLK*(K_CL+K_CH)] float32 per-slot
                localrow / edge value (chunk-major slot layout)
    """
    core = rows // RPC
    local = rows - core * RPC
    blk = local // BLK
    lr_e = (local - blk * BLK).astype(np.float32)
    low = cols < HI

    key = (core * NBLK + blk) * 2 + (~low).astype(np.int64)
    counts = np.bincount(key, minlength=NCORES * NBLK * 2)
    cl = counts[0::2]
    ch = counts[1::2]
    K_CL = max(1, int(np.ceil(cl.max() / 128)))
    K_CH = max(1, int(np.ceil(ch.max() / 128)))
    K_TOT = K_CL + K_CH
    C = NBLK * K_TOT

    # index within (core, block, low/high) group; edges sorted by rows so
    # key groups are contiguous in a stable argsort of key
    order = np.argsort(key, kind="stable")
    starts = np.zeros(NCORES * NBLK * 2, np.int64)
    np.cumsum(counts[:-1], out=starts[1:])
    j = np.empty(len(rows), np.int64)
    j[order] = np.arange(len(rows)) - starts[key[order]]

    # chunk index within the core (chunk-major slot layout)
    sub = np.where(low, j // 128, K_CL + j // 128)
    chunk = blk * K_TOT + sub
    part = j % 128

    lrs = np.zeros((NCORES, 128, C), np.float32)
    valsx = np.zeros((NCORES, 128, C), np.float32)
    lrs[core, part, chunk] = lr_e
    valsx[core, part, chunk] = vals

    # gather streams: position within pass-P stream of a core =
    # (blk*K_CP + j//128)*128 + j%128 ; padding -> index 0
    SL = NBLK * K_CL * 128
    SH = NBLK * K_CH * 128
    sL = np.zeros((NCORES, SL), np.int16)
    sH = np.zeros((NCORES, SH), np.int16)
    posL = (blk[low] * K_CL + j[low] // 128) * 128 + j[low] % 128
    posH = (blk[~low] * K_CH + j[~low] // 128) * 128 + j[~low] % 128
    sL[core[low], posL] = cols[low].astype(np.int16)
    sH[core[~low], posH] = (cols[~low] - HI).astype(np.int16)

    # [16, S/16] wrap (idx i at [i%16, i//16]), replicated to 128 partitions
    # (each of the 8 GPSIMD Q7 cores reads its own 16-partition window)
    idxL = np.tile(sL.reshape(NCORES, SL // 16, 16).transpose(0, 2, 1),
                   (1, 8, 1)).copy()
    idxH = np.tile(sH.reshape(NCORES, SH // 16, 16).transpose(0, 2, 1),
                   (1, 8, 1)).copy()
    return idxL, idxH, lrs, valsx, K_CL, K_CH


def _build(K_CL, K_CH):
    K_TOT = K_CL + K_CH
    C = NBLK * K_TOT
    SL = NBLK * K_CL * 128
    SH = NBLK * K_CH * 128
    nreg = -(-NBLK // BPG)
    nc = bacc.Bacc("TRN2", target_bir_lowering=False, debug=False,
                   num_devices=NCORES, dynamic_dma_scratch_size=65536,
                   num_swdge_queues=4, detect_race_conditions=False)
    f32 = mybir.dt.float32
    x_ap = nc.dram_tensor("x", [N_NODES, D], f32, kind="ExternalInput").ap()
    iL_ap = nc.dram_tensor("idxL", [128, SL // 16], mybir.dt.int16,
                           kind="ExternalInput").ap()
    iH_ap = nc.dram_tensor("idxH", [128, SH // 16], mybir.dt.int16,
                           kind="ExternalInput").ap()
    lr_ap = nc.dram_tensor("lr", [128, C], f32, kind="ExternalInput").ap()
    nlr_ap = nc.dram_tensor("nlr", [128, C], f32, kind="ExternalInput").ap()
    nvals_ap = nc.dram_tensor("nvals", [128, C], f32, kind="ExternalInput").ap()
    vals_ap = nc.dram_tensor("vals", [128, C], f32, kind="ExternalInput").ap()
    iota_ap = nc.dram_tensor("iota", [128, 129], f32, kind="ExternalInput").ap()
    out_ap = nc.dram_tensor("out", [RPC, D], f32, kind="ExternalOutput").ap()
    out_v = out_ap.rearrange("(b p) d -> b p d", p=128)

    with tile.TileContext(nc) as tc:
        with ExitStack() as ctx:
            pp = ctx.enter_context(tc.tile_pool(name="persist", bufs=1))
            gpl = ctx.enter_context(tc.tile_pool(name="gatherL", bufs=GBUFS))
            gph = ctx.enter_context(tc.tile_pool(name="gatherH", bufs=GBUFS))
            sp = ctx.enter_context(tc.tile_pool(name="sbuild", bufs=24))
            ps = ctx.enter_context(tc.tile_pool(name="psum", bufs=8,
                                                space="PSUM"))
            stg = ctx.enter_context(tc.tile_pool(name="stage", bufs=6))
            tp = ctx.enter_context(tc.tile_pool(name="tsq", bufs=12))

            iL_t = pp.tile([128, SL // 16], mybir.dt.int16)
            nc.sync.dma_start(iL_t[:], iL_ap[:])
            iH_t = pp.tile([128, SH // 16], mybir.dt.int16)
            nc.sync.dma_start(iH_t[:], iH_ap[:])
            lr_t = pp.tile([128, C], f32)
            nc.sync.dma_start(lr_t[:], lr_ap[:])
            nlr_t = pp.tile([128, C], f32)
            nc.sync.dma_start(nlr_t[:], nlr_ap[:])
            nvals_t = pp.tile([128, C], f32)
            nc.sync.dma_start(nvals_t[:], nvals_ap[:])
            vals_t = pp.tile([128, C], f32)
            nc.sync.dma_start(vals_t[:], vals_ap[:])
            iota_t = pp.tile([128, 129], f32)
            nc.sync.dma_start(iota_t[:], iota_ap[:])

            gtsL = [None] * nreg
            gtsH = [None] * nreg
            # per-queue DMA-completion sems: completions within one SWDGE
            # queue are in ring order, so per-queue counts are monotone
            qsems = [nc.alloc_semaphore(f"gq{q}") for q in range(4)]
            qcount = [0, 0, 0, 0]
            # (region) -> [(queue, target_count), (queue, target_count)]
            gtarget = [None] * nreg

            def issue_gathers(r):
                # prepare_only + per-queue trigger: Q7 only writes the ring
                # (descriptor gen), the SDMA engines drain 4 queue rings in
                # parallel; data-completion gating is via per-queue sems
                nblk_r = min(BPG, NBLK - r * BPG)
                nL = nblk_r * K_CL * 128
                nH = nblk_r * K_CH * 128
                gtsL[r] = gpl.tile([128, BPG * K_CL, D], f32, name=f"gtL{r}", tag="gtL")
                gtsH[r] = gph.tile([128, BPG * K_CH, D], f32, name=f"gtH{r}", tag="gtH")
                aL = r * BPG * K_CL * 128 // 16
                aH = r * BPG * K_CH * 128 // 16
                q = r % 4
                nc.gpsimd.dma_gather(
                    out_ap=gtsL[r][:, :nblk_r * K_CL, :],
                    in_ap=x_ap[:],
                    idxs_ap=iL_t[:, aL:aL + nL // 16],
                    num_idxs=nL,
                    num_idxs_reg=nL,
                    elem_size=D,
                    single_packet=False,
                    queue_num=q,
                )
                nc.gpsimd.dma_gather(
                    out_ap=gtsH[r][:, :nblk_r * K_CH, :],
                    in_ap=x_ap[HI:, :],
                    idxs_ap=iH_t[:, aH:aH + nH // 16],
                    num_idxs=nH,
                    num_idxs_reg=nH,
                    elem_size=D,
                    single_packet=False,
                    queue_num=q,
                )
                qcount[q] += 1
                gtarget[r] = (q, 32 * qcount[q])

            def do_block(b):
                r, brel = divmod(b, BPG)
                pt = ps.tile([128, 128], f32)
                for k in range(K_TOT):
                    c = b * K_TOT + k
                    if k < K_CL:
                        rhs = gtsL[r][:, brel * K_CL + k, :]
                    else:
                        rhs = gtsH[r][:, brel * K_CH + (k - K_CL), :]
                    s_t = sp.tile([128, 129], f32)
                    if k % 2 == 0:
                        nc.vector.tensor_scalar(
                            s_t[:], iota_t[:], lr_t[:, c:c + 1],
                            vals_t[:, c:c + 1],
                            mybir.AluOpType.is_equal, mybir.AluOpType.mult)
                    else:
                        # ACT: S = Relu(vals*(1 - (iota - lr)^2)), exact for
                        # integer iota-lr and vals >= 0
                        t_t = tp.tile([128, 129], f32, name=f"t{c}", tag="tsq")
                        nc.scalar.activation(
                            t_t[:], iota_t[:],
                            mybir.ActivationFunctionType.Square,
                            bias=nlr_t[:, c:c + 1])
                        nc.scalar.activation(
                            s_t[:], t_t[:],
                            mybir.ActivationFunctionType.Relu,
                            bias=vals_t[:, c:c + 1],
                            scale=nvals_t[:, c:c + 1])
                    mm = nc.tensor.matmul(pt[:], lhsT=s_t[:, :128], rhs=rhs,
                                          start=(k == 0),
                                          stop=(k == K_TOT - 1))

                ot = stg.tile([128, 128], f32)
                nc.scalar.copy(ot[:], pt[:])
                nc.sync.dma_start(out_v[b], ot[:])

            issue_gathers(0)
            issue_gathers(1)
            issue_gathers(2)
            for r in range(nreg):
                if r + 3 < nreg:
                    issue_gathers(r + 3)
                for brel in range(BPG):
                    b = r * BPG + brel
                    if b < NBLK:
                        do_block(b)

    nc.compile()
    return nc


_CACHE = {}


def kernel(x, vals, rows, cols):
    global last_exec_ns
    x = np.ascontiguousarray(np.asarray(x, dtype=np.float32))
    vals = np.asarray(vals, dtype=np.float32)
    rows = np.asarray(rows, dtype=np.int64)
    cols = np.asarray(cols, dtype=np.int64)
    assert x.shape == (N_NODES, D) and vals.shape == rows.shape == cols.shape \
        == (N_EDGES,)

    idxL, idxH, lrs, valsx, K_CL, K_CH = _shard(rows, cols, vals)

    key = (K_CL, K_CH)
    if key not in _CACHE:
        _CACHE[key] = _build(K_CL, K_CH)
    nc = _CACHE[key]

    iota = np.broadcast_to(np.arange(129, dtype=np.float32), (128, 129)).copy()
    in_maps = [
        {"x": x, "idxL": idxL[c], "idxH": idxH[c],
         "lr": lrs[c], "vals": valsx[c], "nlr": -lrs[c], "nvals": -valsx[c],
         "iota": iota}
        for c in range(NCORES)
    ]

    trace = os.environ.get("KERNEL_PROFILE", "0") == "1"
    res = run_bass_kernel_spmd(nc, in_maps, core_ids=list(range(NCORES)),
                               trace=trace)
    last_exec_ns = res.exec_time_ns

    out = np.concatenate([res.results[c]["out"] for c in range(NCORES)],
                         axis=0)
    return out[:N_NODES]
